# revision 23
# baseline (speedup 1.0000x reference)
"""Trainium2 Bass kernel for nn_FFNet_17600775979626.

Spiking FFN layer: cur = einsum('tbi,oi->tbo', x, W) + b followed by a
leaky-integrate-and-fire scan over T (snntorch Leaky, beta=0.95, th=1.0,
subtractive reset). Returns spk_rec [T, B, NO] (0.0/1.0 floats).

Distribution: output-neuron sharding. Each of the 8 cores computes all
(T, B) for a 256-wide slice of the 2048 output neurons.

GEMM precision scheme (validated against the reference on the actual
inputs, ~250/33.5M spike mismatches, rel err ~9e-3 vs the 2e-2 gate):
  x = xh + xl   (xh = fp16 at scale 2^8, xl fp32 residual)
  W = Wh + Wl   (Wh = fp16-rounded W, Wl fp32 residual)
  cur ~= xh@Wh            fp16 matmuls, 1 cyc/row
       + fp8[xl]@fp8[Wh]  fp8e4 DoubleRow matmuls, 0.5 cyc/row
       + fp8[xh]@fp8[Wl]  fp8e4 DoubleRow matmuls, 0.5 cyc/row
All three accumulate into ONE PSUM tile at scale 2^15 (operand scales
2^8*2^7, 2^11*2^4, 2^0*2^15), so the LIF scan runs directly at scale
2^15 (power-of-2 scaling is exact in fp32: bit-identical decisions).
Scale 2^15, not 2^16: the spike value must be fp16-representable
(fp16 max finite is 65504, so 2^16 would saturate to inf).

Layout: W is the PE-stationary operand, x streams FD=512 columns
(4 timesteps x 128 batch, the ISA max) per matmul, so the DoubleRow
256-column LDWEIGHTS (which disables FWL) hides under the previous
matmul's stream. Membrane state lives as [o(part 128) x 2 o-tiles, b]; spikes
are written o-major ([O_S, T, B] fp16 with value 2^15) and transposed
to [T, B, O_S] on the host.

The xl fp8 plane ships from the host (scale 2^11) on the second
HWDGE queue; the fp8(xh) plane is derived on-device with the convert
split between the ACT and DVE engines (one engine alone would become
the bottleneck, and shipping it would push the chip's aggregate HBM
read traffic past ~2.9 TB/s: 8 cores each re-read the replicated x).

Walrus codegen on this target accepts at most ONE sync-wait command per
engine instruction, while Tile's wait assigner freely emits several. Two
post-scheduling passes fix that: _slim_waits drops waits already implied
transitively (per-queue FIFO dispatch + semaphore vector clocks), and
_split_waits moves any excess waits onto injected same-queue NoOps.
"""

import numpy as np

T, B, NI, NO = 128, 128, 2048, 2048
NCORES = 8
O_S = NO // NCORES  # 256 output neurons per core
KC = NI // 128  # 16 fp16 contraction chunks
DRC = NI // 256  # 8 DoubleRow chunks per fp8 gemm (K_eff=256 each)
TQ = 4  # timesteps per group (FD = TQ*B = 512 streamed columns; ISA caps the
        # matmul moving operand at 512 elements per partition)
NG = T // TQ  # 32 groups
BETA = 0.95

# operand scales; all products land in PSUM at scale 2^15
SC_X = 256.0  # xh fp16 plane: x * 2^8
SC_WH = 128.0  # Wh fp16 plane: Wh * 2^7   (2^8 * 2^7 = 2^15)
SC_XL = 2048.0  # xl fp8 plane: (x - xh) * 2^11
SC_WA = 16.0  # Wh fp8 plane: Wh * 2^4   (pairs with xl: 2^11 * 2^4)
SC_WB = 32768.0  # Wl fp8 plane: Wl * 2^15  (pairs with fp8(xh): 2^0 * 2^15)
TH_S = 32768.0  # threshold 1.0 at scale 2^15 (exact in fp16)

_cache = {}


def _build_nc():
    from contextlib import ExitStack

    import concourse.bass as bass
    import concourse.mybir as mybir
    import concourse.tile as tile

    f32 = mybir.dt.float32
    f16 = mybir.dt.float16
    f8 = mybir.dt.float8e4
    DR = mybir.MatmulPerfMode.DoubleRow

    nc = bass.Bass()
    xh = nc.declare_dram_parameter("xh", [NI, T * B], f16, isOutput=False)
    # fp8 xl plane (xl*2^11); the fp8(xh) plane is derived on-device
    xl8 = nc.declare_dram_parameter("xl8", [NI, T * B], f8, isOutput=False)
    # [128, kc, o]: fp16 W^T slice, scale 2^7
    wh = nc.declare_dram_parameter("wh", [128, KC, O_S], f16, isOutput=False)
    # [128, kc', o]: kc' 0..15 = Wh*2^4, 16..31 = Wl*2^15 (DR pair dim inside)
    wc = nc.declare_dram_parameter("wc", [128, 2 * KC, O_S], f8, isOutput=False)
    # bias at scale 2^15, broadcast: bb[p, j*128+col] = b_s[j*128+p]
    bb = nc.declare_dram_parameter("bb", [128, 2 * 128], f32, isOutput=False)
    # spikes o-major, fp16 with value 2^15 (host maps nonzero -> 1.0)
    spk = nc.declare_dram_parameter("spk", [O_S, T, B], f16, isOutput=True)

    with tile.TileContext(nc) as tc, ExitStack() as ctx:
        singles = ctx.enter_context(tc.tile_pool(name="singles", bufs=1))
        xpool = ctx.enter_context(tc.tile_pool(name="xp", bufs=2))
        cpool = ctx.enter_context(tc.tile_pool(name="cp", bufs=3))
        spool = ctx.enter_context(tc.tile_pool(name="sp", bufs=3))
        psum = ctx.enter_context(tc.tile_pool(name="ps", bufs=3, space="PSUM"))

        xhr = xh[:].rearrange("(k p) tb -> p k tb", p=128)
        xcr = xl8[:].rearrange("(k p) tb -> p k tb", p=128)
        spk_r = spk[:].rearrange("(j p) t b -> p j t b", p=128)

        # Startup: W tiles first (small), then group 0's x split across
        # both HWDGE queues, so the first matmuls start ~7us in instead
        # of waiting ~18us for monolithic transfers.
        # Queue order matched to consumption: SP feeds wh + xh quarters 0
        # and 2 (+ wc, needed only ~14us in, after all fp16 matmuls); ACT
        # feeds xh quarters 1 and 3 then the big xc0. First matmul starts
        # ~5us in with no further stalls.
        FD = TQ * B
        Q = KC // 4
        wh_sb = singles.tile([128, KC, O_S], f16)
        nc.sync.dma_start(out=wh_sb[:], in_=wh[:])
        xh0 = xpool.tile([128, KC, FD], f16, tag="xh")
        nc.scalar.dma_start(out=xh0[:, :Q], in_=xhr[:, :Q, :FD])
        nc.sync.dma_start(out=xh0[:, Q : 2 * Q], in_=xhr[:, Q : 2 * Q, :FD])
        nc.scalar.dma_start(out=xh0[:, 2 * Q : 3 * Q], in_=xhr[:, 2 * Q : 3 * Q, :FD])
        nc.sync.dma_start(out=xh0[:, 3 * Q :], in_=xhr[:, 3 * Q :, :FD])
        wc_sb = singles.tile([128, 2 * KC, O_S], f8)
        nc.sync.dma_start(out=wc_sb[:], in_=wc[:])
        xc0 = cpool.tile([128, 2 * KC, FD], f8, tag="xc")
        nc.scalar.dma_start(out=xc0[:, :KC], in_=xcr[:, :, :FD])
        _convert_xh8(nc, mybir, xc0, xh0)

        bfull = singles.tile([128, 2, 128], f32)
        nc.sync.dma_start(out=bfull[:], in_=bb[:].rearrange("p (j c) -> p j c", j=2))

        w_sb = singles.tile([128, 2, 128], f32)  # carry: beta*m - spk + b
        m_sb = singles.tile([128, 2, 128], f32)  # membrane (scaled 2^15)
        wt_sb = singles.tile([128, 2, 128], f32)  # w scratch
        nc.vector.tensor_copy(w_sb[:], bfull[:])  # w_init = b

        # Taper the final groups so the serial scan chain after the very
        # last matmul (the only un-overlappable DVE work) is short.
        group_tqs = [TQ] * (T // TQ - 1) + [TQ // 2, TQ // 2]
        assert sum(group_tqs) == T
        t0 = 0
        for g, tq in enumerate(group_tqs):
            fd = tq * B
            if g == 0:
                xh_t, xc_t = xh0, xc0
            else:
                sl = slice(t0 * B, t0 * B + fd)
                xh_t = xpool.tile([128, KC, fd], f16, tag="xh")
                nc.sync.dma_start(out=xh_t[:], in_=xhr[:, :, sl])
                xc_t = cpool.tile([128, 2 * KC, fd], f8, tag="xc")
                nc.scalar.dma_start(out=xc_t[:, :KC], in_=xcr[:, :, sl])
                _convert_xh8(nc, mybir, xc_t, xh_t)

            # All fp16 matmuls (both PSUM banks) before the fp8 ones: the
            # accumulation group is a per-bank property, and this buys the
            # xc/wc DMAs an extra half-group of slack before the first
            # DoubleRow matmul needs them.
            # The tile is always allocated at the full FD so each j plane
            # stays 2KB-bank-aligned: a matmul `start` zeroes the whole
            # 2KB region around its offset, so a j=1 group starting at a
            # sub-bank offset would wipe half of j=0's accumulation.
            ps_full = psum.tile([128, 2, FD], f32, tag="c", name="ps")
            ps = ps_full[:, :, :fd]
            for j in range(2):
                osl = slice(j * 128, (j + 1) * 128)
                for kc in range(KC):
                    nc.tensor.matmul(
                        ps[:, j, :],
                        lhsT=wh_sb[:, kc, osl],
                        rhs=xh_t[:, kc, :],
                        start=(kc == 0),
                        stop=False,
                    )
            for j in range(2):
                osl = slice(j * 128, (j + 1) * 128)
                for c in range(2 * DRC):
                    nc.tensor.matmul(
                        ps[:, j, :],
                        lhsT=wc_sb[:, 2 * c : 2 * c + 2, osl],
                        rhs=xc_t[:, 2 * c : 2 * c + 2, :],
                        perf_mode=DR,
                        start=False,
                        stop=(c == 2 * DRC - 1),
                    )

            st_t = spool.tile([128, 2, tq, 128], f16, tag="st")
            for tt in range(tq):
                csl = slice(tt * 128, (tt + 1) * 128)
                # m = cur + w   (all at scale 2^15)
                nc.vector.tensor_tensor(
                    m_sb[:], w_sb[:], ps[:, :, csl], mybir.AluOpType.add
                )
                # spk = (m > th) * 2^15  (fp16: 2^15 exact)
                nc.vector.tensor_scalar(
                    st_t[:, :, tt, :], m_sb[:], TH_S, TH_S,
                    mybir.AluOpType.is_gt, mybir.AluOpType.mult,
                )
                # w = beta*m - spk + b
                nc.vector.scalar_tensor_tensor(
                    wt_sb[:], m_sb[:], BETA, st_t[:, :, tt, :],
                    mybir.AluOpType.mult, mybir.AluOpType.subtract,
                )
                nc.vector.tensor_tensor(
                    w_sb[:], wt_sb[:], bfull[:], mybir.AluOpType.add
                )
            nc.sync.dma_start(out=spk_r[:, :, t0 : t0 + tq, :], in_=st_t[:])
            t0 += tq

    _slim_waits(nc)
    _split_waits(nc)
    return nc


def _convert_xh8(nc, mybir, xc_t, xh_t):
    """Fill xc_t[:, KC:, :] with fp8(xh * 2^-8), split across ACT and DVE.

    Shipping this plane from the host would push the chip's aggregate HBM
    read traffic past the ~2.9 TB/s budget (8 cores x replicated x); one
    engine alone doing the convert becomes the bottleneck, so each takes
    half (ACT ~4.4us, DVE scan+half ~9.3us vs the 10.2us PE group).
    """
    H = KC // 2
    nc.scalar.activation(
        out=xc_t[:, KC : KC + H, :], in_=xh_t[:, :H, :],
        func=mybir.ActivationFunctionType.Copy, scale=1.0 / SC_X,
    )
    nc.vector.tensor_scalar(
        xc_t[:, KC + H :, :], xh_t[:, H:, :], 1.0 / SC_X, None,
        mybir.AluOpType.mult,
    )


def _slim_waits(nc):
    """Drop sync waits already implied by earlier ones (transitive closure).

    Each engine queue dispatches in FIFO order, so a wait satisfied on an
    earlier instruction of the same queue covers later instructions. A wait
    on sem s >= v also imports everything the incrementing instruction's
    queue had itself waited for when it raised s to v (semaphore vector
    clocks with snapshots at each increment).
    """
    FRAMEWORK_OPS = ("InstEventSemaphore", "InstDrain")
    engine_clock = {}  # engine -> {sem_id: value known reached}
    totals = {}  # sem_id -> running total of increments
    snapshots = {}  # sem_id -> [(value, clock dict)] in increasing value order
    poisoned = set()  # sems touched by non-monotonic updates (barriers)

    def join(dst, src):
        for s, v in src.items():
            if s in poisoned:
                continue
            if dst.get(s, -1) < v:
                dst[s] = v

    for blk in nc.m.functions[0].blocks:
        for inst in blk.instructions:
            si = getattr(inst, "sync_info", None)
            if si is None:
                continue
            is_framework = type(inst).__name__ in FRAMEWORK_OPS
            clock = engine_clock.setdefault(inst.engine, {})
            if si.on_wait:
                kept = []
                for w in si.on_wait:
                    if (
                        w.sync_type != "semaphore"
                        or w.wait_mode != "sem-ge-imm"
                        or w.id in poisoned
                    ):
                        kept.append(w)
                        continue
                    covered = clock.get(w.id, -1) >= w.wait_value
                    for val, snap in snapshots.get(w.id, ()):
                        if val <= w.wait_value:
                            join(clock, snap)
                        else:
                            break
                    if clock.get(w.id, -1) < w.wait_value:
                        clock[w.id] = w.wait_value
                    if is_framework or not covered:
                        kept.append(w)
                si.on_wait = kept
            if si.on_update:
                for u in si.on_update:
                    if u.sync_type != "semaphore":
                        continue
                    if u.update_mode not in ("sem-inc", "sem-add-imm"):
                        # barrier-style sem: stop reasoning about it entirely
                        poisoned.add(u.id)
                        totals.pop(u.id, None)
                        snapshots.pop(u.id, None)
                        for c in engine_clock.values():
                            c.pop(u.id, None)
                        continue
                    if u.id in poisoned:
                        continue
                    tot = totals.get(u.id, 0) + (u.update_value or 1)
                    totals[u.id] = tot
                    snap = dict(clock)
                    snap[u.id] = tot
                    snapshots.setdefault(u.id, []).append((tot, snap))


def _split_waits(nc, limit=1):
    """Move excess sync waits onto injected same-queue NoOps.

    Walrus codegen accepts at most `limit` sync-wait commands per engine
    instruction on this target. Engine queues dispatch in order, so a
    preceding NoOp carrying the wait is equivalent.
    """
    import concourse.mybir as mybir

    n_nops = 0
    for blk in nc.m.functions[0].blocks:
        out = []
        changed = False
        for inst in blk.instructions:
            si = getattr(inst, "sync_info", None)
            if type(inst).__name__ == "InstEventSemaphore":
                out.append(inst)
                continue
            if si is not None and si.on_wait and len(si.on_wait) > limit:
                waits = list(si.on_wait)
                for w in waits[:-limit]:
                    nop = mybir.InstNoOp(name=f"wnop-{n_nops}", ins=[], outs=[])
                    n_nops += 1
                    nop.engine = inst.engine
                    nop.sync_info = mybir.SyncInfo(on_wait=[w], on_update=[])
                    nop.bass_nofuse = True
                    out.append(nop)
                    changed = True
                si.on_wait = waits[-limit:]
            out.append(inst)
        if changed:
            try:
                blk.instructions = out
            except Exception:
                blk.instructions.clear()
                blk.instructions.extend(out)


def _prepare_in_maps(x, W, b):
    import ml_dtypes

    fp8 = ml_dtypes.float8_e4m3

    x = np.ascontiguousarray(x, dtype=np.float32)
    W = np.ascontiguousarray(W, dtype=np.float32)
    b = np.ascontiguousarray(b, dtype=np.float32)

    x2 = x.reshape(T * B, NI)
    xh16 = (x2 * SC_X).astype(np.float16)
    xh_nat = xh16.astype(np.float32) / SC_X
    xl8 = ((x2 - xh_nat) * SC_XL).astype(fp8)
    xh_dram = np.ascontiguousarray(xh16.T)
    xl8_dram = np.ascontiguousarray(xl8.T)

    Wh16 = (W * SC_WH).astype(np.float16)  # [NO, NI] at 2^7
    Wh_nat = Wh16.astype(np.float32) / SC_WH
    Wl = W - Wh_nat
    wcA = (Wh_nat * SC_WA).astype(fp8)
    wcB = (Wl * SC_WB).astype(fp8)
    b_s = b * TH_S

    in_maps = []
    for c in range(NCORES):
        osl = slice(c * O_S, (c + 1) * O_S)
        # wh: [NI, O_S] -> [kc, 128, O_S] -> [128, kc, O_S]
        wh_host = np.ascontiguousarray(
            Wh16[osl].T.reshape(KC, 128, O_S).transpose(1, 0, 2)
        )
        # wc: [2, NI, O_S] -> [2, kc, 128, O_S] -> [128, 2*kc, O_S]
        wc_full = np.stack([wcA[osl].T, wcB[osl].T])
        wc_host = np.ascontiguousarray(
            wc_full.reshape(2, KC, 128, O_S).transpose(2, 0, 1, 3).reshape(
                128, 2 * KC, O_S
            )
        )
        bs2 = b_s[osl].reshape(2, 128)
        bb_host = np.ascontiguousarray(
            np.broadcast_to(bs2.T[:, :, None], (128, 2, 128)).reshape(128, 256)
        )
        in_maps.append(
            {
                "xh": xh_dram,
                "xl8": xl8_dram,
                "wh": wh_host,
                "wc": wc_host,
                "bb": bb_host,
            }
        )
    return in_maps


def run(x, W, b, trace=False):
    """Run the kernel; returns (out [T,B,NO] fp32, BassKernelResults)."""
    from concourse.bass_utils import run_bass_kernel_spmd

    if "nc" not in _cache:
        _cache["nc"] = _build_nc()
    nc = _cache["nc"]
    in_maps = _prepare_in_maps(x, W, b)
    res = run_bass_kernel_spmd(nc, in_maps, list(range(NCORES)), trace=trace)
    outs = []
    for c in range(NCORES):
        s = res.results[c]["spk"]  # [O_S, T, B] fp16, values {0, 2^15}
        outs.append((s.transpose(1, 2, 0) != 0).astype(np.float32))
    out = np.concatenate(outs, axis=2)
    return out, res


def kernel(x, W, b):
    out, _ = run(x, W, b, trace=False)
    return out


# revision 28
# speedup vs baseline: 1.0047x; 1.0047x over previous
"""Trainium2 Bass kernel for nn_FFNet_17600775979626.

Spiking FFN layer: cur = einsum('tbi,oi->tbo', x, W) + b followed by a
leaky-integrate-and-fire scan over T (snntorch Leaky, beta=0.95, th=1.0,
subtractive reset). Returns spk_rec [T, B, NO] (0.0/1.0 floats).

Distribution: output-neuron sharding. Each of the 8 cores computes all
(T, B) for a 256-wide slice of the 2048 output neurons.

GEMM precision scheme (validated against the reference on the actual
inputs, ~250/33.5M spike mismatches, rel err ~9e-3 vs the 2e-2 gate):
  x = xh + xl   (xh = fp16 at scale 2^8, xl fp32 residual)
  W = Wh + Wl   (Wh = fp16-rounded W, Wl fp32 residual)
  cur ~= xh@Wh            fp16 matmuls, 1 cyc/row
       + fp8[xl]@fp8[Wh]  fp8e4 DoubleRow matmuls, 0.5 cyc/row
       + fp8[xh]@fp8[Wl]  fp8e4 DoubleRow matmuls, 0.5 cyc/row
All three accumulate into ONE PSUM tile at scale 2^15 (operand scales
2^8*2^7, 2^11*2^4, 2^0*2^15), so the LIF scan runs directly at scale
2^15 (power-of-2 scaling is exact in fp32: bit-identical decisions).
Scale 2^15, not 2^16: the spike value must be fp16-representable
(fp16 max finite is 65504, so 2^16 would saturate to inf).

Layout: W is the PE-stationary operand, x streams FD=512 columns
(4 timesteps x 128 batch, the ISA max) per matmul, so the DoubleRow
256-column LDWEIGHTS (which disables FWL) hides under the previous
matmul's stream. Membrane state lives as [o(part 128) x 2 o-tiles, b]; spikes
are written o-major ([O_S, T, B] fp16 with value 2^15) and transposed
to [T, B, O_S] on the host.

The xl fp8 plane ships from the host (scale 2^11) on the second
HWDGE queue; the fp8(xh) plane is derived on-device with the convert
split between the ACT and DVE engines (one engine alone would become
the bottleneck, and shipping it would push the chip's aggregate HBM
read traffic past ~2.9 TB/s: 8 cores each re-read the replicated x).

Walrus codegen on this target accepts at most ONE sync-wait command per
engine instruction, while Tile's wait assigner freely emits several. Two
post-scheduling passes fix that: _slim_waits drops waits already implied
transitively (per-queue FIFO dispatch + semaphore vector clocks), and
_split_waits moves any excess waits onto injected same-queue NoOps.
"""

import numpy as np

T, B, NI, NO = 128, 128, 2048, 2048
NCORES = 8
O_S = NO // NCORES  # 256 output neurons per core
KC = NI // 128  # 16 fp16 contraction chunks
DRC = NI // 256  # 8 DoubleRow chunks per fp8 gemm (K_eff=256 each)
TQ = 4  # timesteps per group (FD = TQ*B = 512 streamed columns; ISA caps the
        # matmul moving operand at 512 elements per partition)
NG = T // TQ  # 32 groups
BETA = 0.95

# operand scales; all products land in PSUM at scale 2^15
SC_X = 256.0  # xh fp16 plane: x * 2^8
SC_WH = 128.0  # Wh fp16 plane: Wh * 2^7   (2^8 * 2^7 = 2^15)
SC_XL = 2048.0  # xl fp8 plane: (x - xh) * 2^11
SC_WA = 16.0  # Wh fp8 plane: Wh * 2^4   (pairs with xl: 2^11 * 2^4)
SC_WB = 32768.0  # Wl fp8 plane: Wl * 2^15  (pairs with fp8(xh): 2^0 * 2^15)
TH_S = 32768.0  # threshold 1.0 at scale 2^15 (exact in fp16)

_cache = {}


def _build_nc():
    from contextlib import ExitStack

    import concourse.bass as bass
    import concourse.mybir as mybir
    import concourse.tile as tile

    f32 = mybir.dt.float32
    f16 = mybir.dt.float16
    f8 = mybir.dt.float8e4
    DR = mybir.MatmulPerfMode.DoubleRow

    nc = bass.Bass()
    xh = nc.declare_dram_parameter("xh", [NI, T * B], f16, isOutput=False)
    # fp8 xl plane (xl*2^11); the fp8(xh) plane is derived on-device
    xl8 = nc.declare_dram_parameter("xl8", [NI, T * B], f8, isOutput=False)
    # [128, kc, o]: fp16 W^T slice, scale 2^7
    wh = nc.declare_dram_parameter("wh", [128, KC, O_S], f16, isOutput=False)
    # [128, kc', o]: kc' 0..15 = Wh*2^4, 16..31 = Wl*2^15 (DR pair dim inside)
    wc = nc.declare_dram_parameter("wc", [128, 2 * KC, O_S], f8, isOutput=False)
    # bias at scale 2^15, broadcast: bb[p, j*128+col] = b_s[j*128+p]
    bb = nc.declare_dram_parameter("bb", [128, 2 * 128], f32, isOutput=False)
    # spikes o-major, fp16 with value 2^15 (host maps nonzero -> 1.0)
    spk = nc.declare_dram_parameter("spk", [O_S, T, B], f16, isOutput=True)

    with tile.TileContext(nc) as tc, ExitStack() as ctx:
        singles = ctx.enter_context(tc.tile_pool(name="singles", bufs=1))
        xpool = ctx.enter_context(tc.tile_pool(name="xp", bufs=2))
        cpool = ctx.enter_context(tc.tile_pool(name="cp", bufs=3))
        spool = ctx.enter_context(tc.tile_pool(name="sp", bufs=3))
        psum = ctx.enter_context(tc.tile_pool(name="ps", bufs=3, space="PSUM"))
        wup = ctx.enter_context(tc.tile_pool(name="wup", bufs=1, space="PSUM"))

        xhr = xh[:].rearrange("(k p) tb -> p k tb", p=128)
        xcr = xl8[:].rearrange("(k p) tb -> p k tb", p=128)
        # (t b) merged so the DMA sees 1KB contiguous runs (a 4D AP
        # leaves 256B innermost runs, which cost 2x in the DMA engines)
        spk_r = spk[:].rearrange("(j p) t b -> p j (t b)", p=128)

        # Startup: W tiles first (small), then group 0's x split across
        # both HWDGE queues, so the first matmuls start ~7us in instead
        # of waiting ~18us for monolithic transfers.
        # Queue order matched to consumption: SP feeds wh + xh quarters 0
        # and 2 (+ wc, needed only ~14us in, after all fp16 matmuls); ACT
        # feeds xh quarters 1 and 3 then the big xc0. First matmul starts
        # ~5us in with no further stalls.
        # PE clock warmup: the tensor engine ramps to full speed only after
        # ~3us of continuous activity (both on hardware and in the cost
        # model), so without this the first ~7 matmuls of group 0 run at
        # half clock. A chain of tiny self-contained matmuls on a memset
        # tile keeps the PE busy from ~0.3us until the x/W DMAs land.
        ones_sb = singles.tile([1, 64], f16)
        nc.vector.memset(ones_sb[:], 1.0)
        ps_w = wup.tile([1, 64], f32)
        for _ in range(52):
            nc.tensor.matmul(
                ps_w[:], lhsT=ones_sb[:, :1], rhs=ones_sb[:], start=True, stop=True
            )

        FD = TQ * B
        Q = KC // 4
        wh_sb = singles.tile([128, KC, O_S], f16)
        nc.sync.dma_start(out=wh_sb[:], in_=wh[:])
        xh0 = xpool.tile([128, KC, FD], f16, tag="xh")
        nc.scalar.dma_start(out=xh0[:, :Q], in_=xhr[:, :Q, :FD])
        nc.sync.dma_start(out=xh0[:, Q : 2 * Q], in_=xhr[:, Q : 2 * Q, :FD])
        nc.scalar.dma_start(out=xh0[:, 2 * Q : 3 * Q], in_=xhr[:, 2 * Q : 3 * Q, :FD])
        nc.sync.dma_start(out=xh0[:, 3 * Q :], in_=xhr[:, 3 * Q :, :FD])
        wc_sb = singles.tile([128, 2 * KC, O_S], f8)
        nc.sync.dma_start(out=wc_sb[:, :KC], in_=wc[:, :KC])
        xc0 = cpool.tile([128, 2 * KC, FD], f8, tag="xc")
        nc.scalar.dma_start(out=xc0[:, :KC], in_=xcr[:, :, :FD])
        nc.scalar.dma_start(out=wc_sb[:, KC:], in_=wc[:, KC:])
        _convert_xh8(nc, mybir, xc0, xh0)

        bfull = singles.tile([128, 2, 128], f32)
        nc.sync.dma_start(out=bfull[:], in_=bb[:].rearrange("p (j c) -> p j c", j=2))

        w_sb = singles.tile([128, 2, 128], f32)  # carry: beta*m - spk + b
        m_sb = singles.tile([128, 2, 128], f32)  # membrane (scaled 2^15)
        wt_sb = singles.tile([128, 2, 128], f32)  # w scratch
        nc.vector.tensor_copy(w_sb[:], bfull[:])  # w_init = b

        # Taper the final groups so the serial scan chain after the very
        # last matmul (the only un-overlappable DVE work) is short.
        group_tqs = [TQ] * (T // TQ - 1) + [TQ // 2, TQ // 2]
        assert sum(group_tqs) == T
        t0 = 0
        for g, tq in enumerate(group_tqs):
            fd = tq * B
            if g == 0:
                xh_t, xc_t = xh0, xc0
            else:
                sl = slice(t0 * B, t0 * B + fd)
                xh_t = xpool.tile([128, KC, fd], f16, tag="xh")
                nc.sync.dma_start(out=xh_t[:], in_=xhr[:, :, sl])
                xc_t = cpool.tile([128, 2 * KC, fd], f8, tag="xc")
                nc.scalar.dma_start(out=xc_t[:, :KC], in_=xcr[:, :, sl])
                _convert_xh8(nc, mybir, xc_t, xh_t)

            # All fp16 matmuls (both PSUM banks) before the fp8 ones: the
            # accumulation group is a per-bank property, and this buys the
            # xc/wc DMAs an extra half-group of slack before the first
            # DoubleRow matmul needs them.
            # The tile is always allocated at the full FD so each j plane
            # stays 2KB-bank-aligned: a matmul `start` zeroes the whole
            # 2KB region around its offset, so a j=1 group starting at a
            # sub-bank offset would wipe half of j=0's accumulation.
            ps_full = psum.tile([128, 2, FD], f32, tag="c", name="ps")
            ps = ps_full[:, :, :fd]
            for j in range(2):
                osl = slice(j * 128, (j + 1) * 128)
                for kc in range(KC):
                    nc.tensor.matmul(
                        ps[:, j, :],
                        lhsT=wh_sb[:, kc, osl],
                        rhs=xh_t[:, kc, :],
                        start=(kc == 0),
                        stop=False,
                    )
            for j in range(2):
                osl = slice(j * 128, (j + 1) * 128)
                for c in range(2 * DRC):
                    nc.tensor.matmul(
                        ps[:, j, :],
                        lhsT=wc_sb[:, 2 * c : 2 * c + 2, osl],
                        rhs=xc_t[:, 2 * c : 2 * c + 2, :],
                        perf_mode=DR,
                        start=False,
                        stop=(c == 2 * DRC - 1),
                    )

            st_t = spool.tile([128, 2, tq, 128], f16, tag="st")
            for tt in range(tq):
                csl = slice(tt * 128, (tt + 1) * 128)
                # m = cur + w   (all at scale 2^15)
                nc.vector.tensor_tensor(
                    m_sb[:], w_sb[:], ps[:, :, csl], mybir.AluOpType.add
                )
                # spk = (m > th) * 2^15  (fp16: 2^15 exact)
                nc.vector.tensor_scalar(
                    st_t[:, :, tt, :], m_sb[:], TH_S, TH_S,
                    mybir.AluOpType.is_gt, mybir.AluOpType.mult,
                )
                # w = beta*m - spk + b (dead after the final timestep,
                # and it sits on the end-of-kernel critical path)
                if not (g == len(group_tqs) - 1 and tt == tq - 1):
                    nc.vector.scalar_tensor_tensor(
                        wt_sb[:], m_sb[:], BETA, st_t[:, :, tt, :],
                        mybir.AluOpType.mult, mybir.AluOpType.subtract,
                    )
                    nc.vector.tensor_tensor(
                        w_sb[:], wt_sb[:], bfull[:], mybir.AluOpType.add
                    )
            nc.sync.dma_start(
                out=spk_r[:, :, t0 * B : (t0 + tq) * B],
                in_=st_t[:].rearrange("p j t b -> p j (t b)"),
            )
            t0 += tq

    _slim_waits(nc)
    _split_waits(nc)
    return nc


def _convert_xh8(nc, mybir, xc_t, xh_t):
    """Fill xc_t[:, KC:, :] with fp8(xh * 2^-8), split across ACT and DVE.

    Shipping this plane from the host would push the chip's aggregate HBM
    read traffic past the ~2.9 TB/s budget (8 cores x replicated x); one
    engine alone doing the convert becomes the bottleneck, so each takes
    half (ACT ~4.4us, DVE scan+half ~9.3us vs the 10.2us PE group).
    """
    H = KC // 2
    nc.scalar.activation(
        out=xc_t[:, KC : KC + H, :], in_=xh_t[:, :H, :],
        func=mybir.ActivationFunctionType.Copy, scale=1.0 / SC_X,
    )
    nc.vector.tensor_scalar(
        xc_t[:, KC + H :, :], xh_t[:, H:, :], 1.0 / SC_X, None,
        mybir.AluOpType.mult,
    )


def _slim_waits(nc):
    """Drop sync waits already implied by earlier ones (transitive closure).

    Each engine queue dispatches in FIFO order, so a wait satisfied on an
    earlier instruction of the same queue covers later instructions. A wait
    on sem s >= v also imports everything the incrementing instruction's
    queue had itself waited for when it raised s to v (semaphore vector
    clocks with snapshots at each increment).
    """
    FRAMEWORK_OPS = ("InstEventSemaphore", "InstDrain")
    engine_clock = {}  # engine -> {sem_id: value known reached}
    totals = {}  # sem_id -> running total of increments
    snapshots = {}  # sem_id -> [(value, clock dict)] in increasing value order
    poisoned = set()  # sems touched by non-monotonic updates (barriers)

    def join(dst, src):
        for s, v in src.items():
            if s in poisoned:
                continue
            if dst.get(s, -1) < v:
                dst[s] = v

    for blk in nc.m.functions[0].blocks:
        for inst in blk.instructions:
            si = getattr(inst, "sync_info", None)
            if si is None:
                continue
            is_framework = type(inst).__name__ in FRAMEWORK_OPS
            clock = engine_clock.setdefault(inst.engine, {})
            if si.on_wait:
                kept = []
                for w in si.on_wait:
                    if (
                        w.sync_type != "semaphore"
                        or w.wait_mode != "sem-ge-imm"
                        or w.id in poisoned
                    ):
                        kept.append(w)
                        continue
                    covered = clock.get(w.id, -1) >= w.wait_value
                    for val, snap in snapshots.get(w.id, ()):
                        if val <= w.wait_value:
                            join(clock, snap)
                        else:
                            break
                    if clock.get(w.id, -1) < w.wait_value:
                        clock[w.id] = w.wait_value
                    if is_framework or not covered:
                        kept.append(w)
                si.on_wait = kept
            if si.on_update:
                for u in si.on_update:
                    if u.sync_type != "semaphore":
                        continue
                    if u.update_mode not in ("sem-inc", "sem-add-imm"):
                        # barrier-style sem: stop reasoning about it entirely
                        poisoned.add(u.id)
                        totals.pop(u.id, None)
                        snapshots.pop(u.id, None)
                        for c in engine_clock.values():
                            c.pop(u.id, None)
                        continue
                    if u.id in poisoned:
                        continue
                    tot = totals.get(u.id, 0) + (u.update_value or 1)
                    totals[u.id] = tot
                    snap = dict(clock)
                    snap[u.id] = tot
                    snapshots.setdefault(u.id, []).append((tot, snap))


def _split_waits(nc, limit=1):
    """Move excess sync waits onto injected same-queue NoOps.

    Walrus codegen accepts at most `limit` sync-wait commands per engine
    instruction on this target. Engine queues dispatch in order, so a
    preceding NoOp carrying the wait is equivalent.
    """
    import concourse.mybir as mybir

    n_nops = 0
    for blk in nc.m.functions[0].blocks:
        out = []
        changed = False
        for inst in blk.instructions:
            si = getattr(inst, "sync_info", None)
            if type(inst).__name__ == "InstEventSemaphore":
                out.append(inst)
                continue
            if si is not None and si.on_wait and len(si.on_wait) > limit:
                waits = list(si.on_wait)
                for w in waits[:-limit]:
                    nop = mybir.InstNoOp(name=f"wnop-{n_nops}", ins=[], outs=[])
                    n_nops += 1
                    nop.engine = inst.engine
                    nop.sync_info = mybir.SyncInfo(on_wait=[w], on_update=[])
                    nop.bass_nofuse = True
                    out.append(nop)
                    changed = True
                si.on_wait = waits[-limit:]
            out.append(inst)
        if changed:
            try:
                blk.instructions = out
            except Exception:
                blk.instructions.clear()
                blk.instructions.extend(out)


def _prepare_in_maps(x, W, b):
    import ml_dtypes

    fp8 = ml_dtypes.float8_e4m3

    x = np.ascontiguousarray(x, dtype=np.float32)
    W = np.ascontiguousarray(W, dtype=np.float32)
    b = np.ascontiguousarray(b, dtype=np.float32)

    x2 = x.reshape(T * B, NI)
    xh16 = (x2 * SC_X).astype(np.float16)
    xh_nat = xh16.astype(np.float32) / SC_X
    xl8 = ((x2 - xh_nat) * SC_XL).astype(fp8)
    xh_dram = np.ascontiguousarray(xh16.T)
    xl8_dram = np.ascontiguousarray(xl8.T)

    Wh16 = (W * SC_WH).astype(np.float16)  # [NO, NI] at 2^7
    Wh_nat = Wh16.astype(np.float32) / SC_WH
    Wl = W - Wh_nat
    wcA = (Wh_nat * SC_WA).astype(fp8)
    wcB = (Wl * SC_WB).astype(fp8)
    b_s = b * TH_S

    in_maps = []
    for c in range(NCORES):
        osl = slice(c * O_S, (c + 1) * O_S)
        # wh: [NI, O_S] -> [kc, 128, O_S] -> [128, kc, O_S]
        wh_host = np.ascontiguousarray(
            Wh16[osl].T.reshape(KC, 128, O_S).transpose(1, 0, 2)
        )
        # wc: [2, NI, O_S] -> [2, kc, 128, O_S] -> [128, 2*kc, O_S]
        wc_full = np.stack([wcA[osl].T, wcB[osl].T])
        wc_host = np.ascontiguousarray(
            wc_full.reshape(2, KC, 128, O_S).transpose(2, 0, 1, 3).reshape(
                128, 2 * KC, O_S
            )
        )
        bs2 = b_s[osl].reshape(2, 128)
        bb_host = np.ascontiguousarray(
            np.broadcast_to(bs2.T[:, :, None], (128, 2, 128)).reshape(128, 256)
        )
        in_maps.append(
            {
                "xh": xh_dram,
                "xl8": xl8_dram,
                "wh": wh_host,
                "wc": wc_host,
                "bb": bb_host,
            }
        )
    return in_maps


def run(x, W, b, trace=False):
    """Run the kernel; returns (out [T,B,NO] fp32, BassKernelResults)."""
    from concourse.bass_utils import run_bass_kernel_spmd

    if "nc" not in _cache:
        _cache["nc"] = _build_nc()
    nc = _cache["nc"]
    in_maps = _prepare_in_maps(x, W, b)
    res = run_bass_kernel_spmd(nc, in_maps, list(range(NCORES)), trace=trace)
    outs = []
    for c in range(NCORES):
        s = res.results[c]["spk"]  # [O_S, T, B] fp16, values {0, 2^15}
        outs.append((s.transpose(1, 2, 0) != 0).astype(np.float32))
    out = np.concatenate(outs, axis=2)
    return out, res


def kernel(x, W, b):
    out, _ = run(x, W, b, trace=False)
    return out
